# revision 5
# baseline (speedup 1.0000x reference)
"""Trainium2 kernel for cellpose-style flow integration (grid_sample scan).

Strategy:
  - Host builds a padded "patch table" T[r*2050+c] = the 8 values
    [a00,a01,a10,a11,b00,b01,b10,b11] of the 2x2 bilinear corner patch at
    padded pixel (r,c); channels a=im[0] (adds to pt x), b=im[1] (adds to
    pt y).  Zero padding rows/cols encode grid_sample's zeros-padding.
  - Points are sharded across 8 NeuronCores (32768 each, laid out [128,256]).
  - State is kept in reference pt coords [-1,1] (fp32 rounding matches the
    reference update).  Each iteration: u = pt*1024+1024.5 (the padded
    sample coordinate), robust floor, q = yf*2050+xf, per-partition
    indirect-DMA patch gathers from HBM, separable bilinear lerp on DVE,
    pt += sample, clip.  Chunked so chunk A's gathers overlap chunk B's
    vector ops.
"""
import numpy as np

H = W = 2048
NPTS = 262144
N_CORES = 8
PTS_PER_CORE = NPTS // N_CORES          # 32768
P = 128
F = PTS_PER_CORE // P                   # 256 free elems per partition
PAD = 2050                              # padded table row length
NCHUNK = 2

_compiled = {}


def _build_nc(niter: int):
    import concourse.bass as bass
    import concourse.mybir as mybir
    import concourse.tile as tile
    from concourse import bacc

    f32 = mybir.dt.float32
    i32 = mybir.dt.int32
    Alu = mybir.AluOpType

    nc = bacc.Bacc("TRN2", target_bir_lowering=False, debug=False,
                   num_devices=N_CORES)
    tab = nc.dram_tensor("tab", [PAD * PAD, 8], f32, kind="ExternalInput").ap()
    p0x = nc.dram_tensor("p0x", [P, F], f32, kind="ExternalInput").ap()
    p0y = nc.dram_tensor("p0y", [P, F], f32, kind="ExternalInput").ap()
    outx = nc.dram_tensor("outx", [P, F], f32, kind="ExternalOutput").ap()
    outy = nc.dram_tensor("outy", [P, F], f32, kind="ExternalOutput").ap()

    FC = F // NCHUNK

    with tile.TileContext(nc) as tc:
        with (
            tc.tile_pool(name="state", bufs=1) as state,
            tc.tile_pool(name="scratch", bufs=3) as scratch,
            tc.tile_pool(name="gbuf", bufs=3) as gbuf,
        ):
            px = state.tile([P, F], f32, tag="px")
            py = state.tile([P, F], f32, tag="py")
            nc.gpsimd.dma_start(out=px[:], in_=p0x[:])
            nc.gpsimd.dma_start(out=py[:], in_=p0y[:])

            for it in range(niter):
                for c in range(NCHUNK):
                    cs = slice(c * FC, (c + 1) * FC)
                    pxc = px[:, cs]
                    pyc = py[:, cs]

                    fx = scratch.tile([P, FC], f32, tag="fx")
                    fy = scratch.tile([P, FC], f32, tag="fy")
                    xf = scratch.tile([P, FC], f32, tag="xf")
                    yf = scratch.tile([P, FC], f32, tag="yf")
                    qf = scratch.tile([P, FC], f32, tag="qf")
                    qi = scratch.tile([P, FC], i32, tag="qi")
                    ti = scratch.tile([P, FC], i32, tag="ti")
                    m = scratch.tile([P, FC], f32, tag="m")
                    uu = scratch.tile([P, FC], f32, tag="uu")

                    # u = pt*1024 + 1024.5 (padded sample coord);
                    # floor robust to int-convert rounding mode:
                    #   xf = int(u - 0.5); fx = u - xf; if fx >= 1: fx -= 1, xf += 1
                    for (pc, fr, fl) in ((pxc, fx, xf), (pyc, fy, yf)):
                        nc.vector.tensor_scalar(out=uu[:], in0=pc,
                                                scalar1=1024.0, scalar2=1024.5,
                                                op0=Alu.mult, op1=Alu.add)
                        nc.vector.tensor_scalar(out=fr[:], in0=uu[:],
                                                scalar1=0.5, scalar2=None,
                                                op0=Alu.subtract)
                        nc.vector.tensor_copy(out=ti[:], in_=fr[:])
                        nc.vector.tensor_copy(out=fl[:], in_=ti[:])
                        nc.vector.tensor_tensor(out=fr[:], in0=uu[:], in1=fl[:],
                                                op=Alu.subtract)
                        nc.vector.tensor_scalar(out=m[:], in0=fr[:], scalar1=1.0,
                                                scalar2=None, op0=Alu.is_ge)
                        nc.vector.tensor_tensor(out=fr[:], in0=fr[:], in1=m[:],
                                                op=Alu.subtract)
                        nc.vector.tensor_tensor(out=fl[:], in0=fl[:], in1=m[:],
                                                op=Alu.add)
                    # qf = yf * 2050 + xf
                    nc.vector.tensor_scalar(out=qf[:], in0=yf[:], scalar1=2050.0,
                                            scalar2=None, op0=Alu.mult)
                    nc.vector.tensor_tensor(out=qf[:], in0=qf[:], in1=xf[:],
                                            op=Alu.add)
                    nc.vector.tensor_copy(out=qi[:], in_=qf[:])

                    g = gbuf.tile([P, FC, 8], f32, tag="g")
                    for j in range(FC):
                        nc.gpsimd.indirect_dma_start(
                            out=g[:, j, :],
                            out_offset=None,
                            in_=tab[:, :],
                            in_offset=bass.IndirectOffsetOnAxis(
                                ap=qi[:, j:j + 1], axis=0),
                        )

                    # x-lerp: h = g_even + fx * (g_odd - g_even)
                    d = scratch.tile([P, FC, 4], f32, tag="d")
                    h = scratch.tile([P, FC, 4], f32, tag="h")
                    nc.vector.tensor_tensor(out=d[:], in0=g[:, :, 1::2],
                                            in1=g[:, :, 0::2], op=Alu.subtract)
                    nc.vector.tensor_tensor(out=d[:], in0=d[:],
                                            in1=fx[:].to_broadcast([P, FC, 4]),
                                            op=Alu.mult)
                    nc.vector.tensor_tensor(out=h[:], in0=g[:, :, 0::2],
                                            in1=d[:], op=Alu.add)
                    # y-lerp: s = h_even + fy * (h_odd - h_even)
                    d2 = scratch.tile([P, FC, 2], f32, tag="d2")
                    s = scratch.tile([P, FC, 2], f32, tag="s")
                    nc.vector.tensor_tensor(out=d2[:], in0=h[:, :, 1::2],
                                            in1=h[:, :, 0::2], op=Alu.subtract)
                    nc.vector.tensor_tensor(out=d2[:], in0=d2[:],
                                            in1=fy[:].to_broadcast([P, FC, 2]),
                                            op=Alu.mult)
                    nc.vector.tensor_tensor(out=s[:], in0=h[:, :, 0::2],
                                            in1=d2[:], op=Alu.add)

                    # pt += s ; clip to [-1, 1]
                    nc.vector.tensor_tensor(out=pxc, in0=pxc, in1=s[:, :, 0],
                                            op=Alu.add)
                    nc.vector.tensor_tensor(out=pyc, in0=pyc, in1=s[:, :, 1],
                                            op=Alu.add)
                    nc.vector.tensor_scalar(out=pxc, in0=pxc, scalar1=-1.0,
                                            scalar2=1.0, op0=Alu.max,
                                            op1=Alu.min)
                    nc.vector.tensor_scalar(out=pyc, in0=pyc, scalar1=-1.0,
                                            scalar2=1.0, op0=Alu.max,
                                            op1=Alu.min)

            # final: pix = (pt + 1) * 1023.5
            ox = state.tile([P, F], f32, tag="ox")
            oy = state.tile([P, F], f32, tag="oy")
            nc.vector.tensor_scalar(out=ox[:], in0=px[:], scalar1=1.0,
                                    scalar2=1023.5, op0=Alu.add, op1=Alu.mult)
            nc.vector.tensor_scalar(out=oy[:], in0=py[:], scalar1=1.0,
                                    scalar2=1023.5, op0=Alu.add, op1=Alu.mult)
            nc.gpsimd.dma_start(out=outx[:], in_=ox[:])
            nc.gpsimd.dma_start(out=outy[:], in_=oy[:])

    nc.compile()
    return nc


def _build_table(dP: np.ndarray) -> np.ndarray:
    """T[r*2050+c, 0:8] = 2x2 patch of (im0,im1) at padded (r,c)."""
    scale = np.float32(2.0 / 2047.0)
    im0 = (dP[1] * scale).astype(np.float32)   # adds to pt x
    im1 = (dP[0] * scale).astype(np.float32)   # adds to pt y
    imp = np.zeros((PAD + 1, PAD + 1, 2), np.float32)
    imp[1:H + 1, 1:W + 1, 0] = im0
    imp[1:H + 1, 1:W + 1, 1] = im1
    T = np.empty((PAD, PAD, 8), np.float32)
    T[:, :, 0] = imp[:PAD, :PAD, 0]       # a00
    T[:, :, 1] = imp[:PAD, 1:, 0]         # a01
    T[:, :, 2] = imp[1:, :PAD, 0]         # a10
    T[:, :, 3] = imp[1:, 1:, 0]           # a11
    T[:, :, 4] = imp[:PAD, :PAD, 1]       # b00
    T[:, :, 5] = imp[:PAD, 1:, 1]         # b01
    T[:, :, 6] = imp[1:, :PAD, 1]         # b10
    T[:, :, 7] = imp[1:, 1:, 1]           # b11
    return T.reshape(PAD * PAD, 8)


def _initial_pts(inds: np.ndarray):
    f = np.float32
    sizes = f(2047.0)
    ptx = inds[1].astype(f) / sizes * f(2.0) - f(1.0)
    pty = inds[0].astype(f) / sizes * f(2.0) - f(1.0)
    return ptx, pty


def kernel(dP: np.ndarray, inds: np.ndarray, niter) -> np.ndarray:
    from concourse.bass_utils import run_bass_kernel_spmd

    niter = int(niter)
    dP = np.asarray(dP, np.float32)
    inds = np.asarray(inds)

    if niter not in _compiled:
        _compiled[niter] = _build_nc(niter)
    nc = _compiled[niter]

    T = _build_table(dP)
    ptx, pty = _initial_pts(inds)

    in_maps = []
    for i in range(N_CORES):
        sl = slice(i * PTS_PER_CORE, (i + 1) * PTS_PER_CORE)
        in_maps.append({
            "tab": T,
            "p0x": ptx[sl].reshape(P, F),
            "p0y": pty[sl].reshape(P, F),
        })

    res = run_bass_kernel_spmd(nc, in_maps, list(range(N_CORES)))

    out = np.empty((2, NPTS), np.float32)
    for i in range(N_CORES):
        sl = slice(i * PTS_PER_CORE, (i + 1) * PTS_PER_CORE)
        out[0, sl] = res.results[i]["outy"].reshape(-1)
        out[1, sl] = res.results[i]["outx"].reshape(-1)
    return out


# revision 6
# speedup vs baseline: 1.1216x; 1.1216x over previous
"""Trainium2 kernel for cellpose-style flow integration (grid_sample scan).

Strategy:
  - Host builds a padded "patch table" T[r*2050+c] = the 8 values
    [a00,a01,a10,a11,b00,b01,b10,b11] of the 2x2 bilinear corner patch at
    padded pixel (r,c); channels a=im[0] (adds to pt x), b=im[1] (adds to
    pt y).  Zero padding rows/cols encode grid_sample's zeros-padding.
  - Points are sharded across 8 NeuronCores (32768 each, laid out [128,256]).
  - State is kept in reference pt coords [-1,1] (fp32 rounding matches the
    reference update).  Each iteration: u = pt*1024+1024.5 (the padded
    sample coordinate), robust floor, q = yf*2050+xf, per-partition
    indirect-DMA patch gathers from HBM, separable bilinear lerp on DVE,
    pt += sample, clip.  Chunked so chunk A's gathers overlap chunk B's
    vector ops.
"""
import numpy as np

H = W = 2048
NPTS = 262144
N_CORES = 8
PTS_PER_CORE = NPTS // N_CORES          # 32768
P = 128
F = PTS_PER_CORE // P                   # 256 free elems per partition
PAD = 2050                              # padded table row length
NCHUNK = 4

_compiled = {}


def _build_nc(niter: int):
    import concourse.bass as bass
    import concourse.mybir as mybir
    import concourse.tile as tile
    from concourse import bacc

    f32 = mybir.dt.float32
    i32 = mybir.dt.int32
    Alu = mybir.AluOpType

    nc = bacc.Bacc("TRN2", target_bir_lowering=False, debug=False,
                   num_devices=N_CORES)
    tab = nc.dram_tensor("tab", [PAD * PAD, 8], f32, kind="ExternalInput").ap()
    p0x = nc.dram_tensor("p0x", [P, F], f32, kind="ExternalInput").ap()
    p0y = nc.dram_tensor("p0y", [P, F], f32, kind="ExternalInput").ap()
    outx = nc.dram_tensor("outx", [P, F], f32, kind="ExternalOutput").ap()
    outy = nc.dram_tensor("outy", [P, F], f32, kind="ExternalOutput").ap()

    FC = F // NCHUNK

    with tile.TileContext(nc) as tc:
        with (
            tc.tile_pool(name="state", bufs=1) as state,
            tc.tile_pool(name="scratch", bufs=4) as scratch,
            tc.tile_pool(name="gbuf", bufs=4) as gbuf,
        ):
            px = state.tile([P, F], f32, tag="px")
            py = state.tile([P, F], f32, tag="py")
            nc.gpsimd.dma_start(out=px[:], in_=p0x[:])
            nc.gpsimd.dma_start(out=py[:], in_=p0y[:])

            for it in range(niter):
                for c in range(NCHUNK):
                    cs = slice(c * FC, (c + 1) * FC)
                    pxc = px[:, cs]
                    pyc = py[:, cs]

                    fx = scratch.tile([P, FC], f32, tag="fx")
                    fy = scratch.tile([P, FC], f32, tag="fy")
                    xf = scratch.tile([P, FC], f32, tag="xf")
                    yf = scratch.tile([P, FC], f32, tag="yf")
                    qf = scratch.tile([P, FC], f32, tag="qf")
                    qi = scratch.tile([P, FC], i32, tag="qi")
                    ti = scratch.tile([P, FC], i32, tag="ti")
                    m = scratch.tile([P, FC], f32, tag="m")
                    uu = scratch.tile([P, FC], f32, tag="uu")

                    # u = pt*1024 + 1024.5 (padded sample coord);
                    # floor robust to int-convert rounding mode:
                    #   xf = int(u - 0.5); fx = u - xf; if fx >= 1: fx -= 1, xf += 1
                    for (pc, fr, fl) in ((pxc, fx, xf), (pyc, fy, yf)):
                        nc.vector.tensor_scalar(out=uu[:], in0=pc,
                                                scalar1=1024.0, scalar2=1024.5,
                                                op0=Alu.mult, op1=Alu.add)
                        nc.vector.tensor_scalar(out=fr[:], in0=uu[:],
                                                scalar1=0.5, scalar2=None,
                                                op0=Alu.subtract)
                        nc.vector.tensor_copy(out=ti[:], in_=fr[:])
                        nc.vector.tensor_copy(out=fl[:], in_=ti[:])
                        nc.vector.tensor_tensor(out=fr[:], in0=uu[:], in1=fl[:],
                                                op=Alu.subtract)
                        nc.vector.tensor_scalar(out=m[:], in0=fr[:], scalar1=1.0,
                                                scalar2=None, op0=Alu.is_ge)
                        nc.vector.tensor_tensor(out=fr[:], in0=fr[:], in1=m[:],
                                                op=Alu.subtract)
                        nc.vector.tensor_tensor(out=fl[:], in0=fl[:], in1=m[:],
                                                op=Alu.add)
                    # qf = yf * 2050 + xf
                    nc.vector.tensor_scalar(out=qf[:], in0=yf[:], scalar1=2050.0,
                                            scalar2=None, op0=Alu.mult)
                    nc.vector.tensor_tensor(out=qf[:], in0=qf[:], in1=xf[:],
                                            op=Alu.add)
                    nc.vector.tensor_copy(out=qi[:], in_=qf[:])

                    g = gbuf.tile([P, FC, 8], f32, tag="g")
                    for j in range(FC):
                        nc.gpsimd.indirect_dma_start(
                            out=g[:, j, :],
                            out_offset=None,
                            in_=tab[:, :],
                            in_offset=bass.IndirectOffsetOnAxis(
                                ap=qi[:, j:j + 1], axis=0),
                        )

                    # x-lerp: h = g_even + fx * (g_odd - g_even)
                    d = scratch.tile([P, FC, 4], f32, tag="d")
                    h = scratch.tile([P, FC, 4], f32, tag="h")
                    nc.vector.tensor_tensor(out=d[:], in0=g[:, :, 1::2],
                                            in1=g[:, :, 0::2], op=Alu.subtract)
                    nc.vector.tensor_tensor(out=d[:], in0=d[:],
                                            in1=fx[:].to_broadcast([P, FC, 4]),
                                            op=Alu.mult)
                    nc.vector.tensor_tensor(out=h[:], in0=g[:, :, 0::2],
                                            in1=d[:], op=Alu.add)
                    # y-lerp: s = h_even + fy * (h_odd - h_even)
                    d2 = scratch.tile([P, FC, 2], f32, tag="d2")
                    s = scratch.tile([P, FC, 2], f32, tag="s")
                    nc.vector.tensor_tensor(out=d2[:], in0=h[:, :, 1::2],
                                            in1=h[:, :, 0::2], op=Alu.subtract)
                    nc.vector.tensor_tensor(out=d2[:], in0=d2[:],
                                            in1=fy[:].to_broadcast([P, FC, 2]),
                                            op=Alu.mult)
                    nc.vector.tensor_tensor(out=s[:], in0=h[:, :, 0::2],
                                            in1=d2[:], op=Alu.add)

                    # pt += s ; clip to [-1, 1]
                    nc.vector.tensor_tensor(out=pxc, in0=pxc, in1=s[:, :, 0],
                                            op=Alu.add)
                    nc.vector.tensor_tensor(out=pyc, in0=pyc, in1=s[:, :, 1],
                                            op=Alu.add)
                    nc.vector.tensor_scalar(out=pxc, in0=pxc, scalar1=-1.0,
                                            scalar2=1.0, op0=Alu.max,
                                            op1=Alu.min)
                    nc.vector.tensor_scalar(out=pyc, in0=pyc, scalar1=-1.0,
                                            scalar2=1.0, op0=Alu.max,
                                            op1=Alu.min)

            # final: pix = (pt + 1) * 1023.5
            ox = state.tile([P, F], f32, tag="ox")
            oy = state.tile([P, F], f32, tag="oy")
            nc.vector.tensor_scalar(out=ox[:], in0=px[:], scalar1=1.0,
                                    scalar2=1023.5, op0=Alu.add, op1=Alu.mult)
            nc.vector.tensor_scalar(out=oy[:], in0=py[:], scalar1=1.0,
                                    scalar2=1023.5, op0=Alu.add, op1=Alu.mult)
            nc.gpsimd.dma_start(out=outx[:], in_=ox[:])
            nc.gpsimd.dma_start(out=outy[:], in_=oy[:])

    nc.compile()
    return nc


def _build_table(dP: np.ndarray) -> np.ndarray:
    """T[r*2050+c, 0:8] = 2x2 patch of (im0,im1) at padded (r,c)."""
    scale = np.float32(2.0 / 2047.0)
    im0 = (dP[1] * scale).astype(np.float32)   # adds to pt x
    im1 = (dP[0] * scale).astype(np.float32)   # adds to pt y
    imp = np.zeros((PAD + 1, PAD + 1, 2), np.float32)
    imp[1:H + 1, 1:W + 1, 0] = im0
    imp[1:H + 1, 1:W + 1, 1] = im1
    T = np.empty((PAD, PAD, 8), np.float32)
    T[:, :, 0] = imp[:PAD, :PAD, 0]       # a00
    T[:, :, 1] = imp[:PAD, 1:, 0]         # a01
    T[:, :, 2] = imp[1:, :PAD, 0]         # a10
    T[:, :, 3] = imp[1:, 1:, 0]           # a11
    T[:, :, 4] = imp[:PAD, :PAD, 1]       # b00
    T[:, :, 5] = imp[:PAD, 1:, 1]         # b01
    T[:, :, 6] = imp[1:, :PAD, 1]         # b10
    T[:, :, 7] = imp[1:, 1:, 1]           # b11
    return T.reshape(PAD * PAD, 8)


def _initial_pts(inds: np.ndarray):
    f = np.float32
    sizes = f(2047.0)
    ptx = inds[1].astype(f) / sizes * f(2.0) - f(1.0)
    pty = inds[0].astype(f) / sizes * f(2.0) - f(1.0)
    return ptx, pty


def kernel(dP: np.ndarray, inds: np.ndarray, niter) -> np.ndarray:
    from concourse.bass_utils import run_bass_kernel_spmd

    niter = int(niter)
    dP = np.asarray(dP, np.float32)
    inds = np.asarray(inds)

    if niter not in _compiled:
        _compiled[niter] = _build_nc(niter)
    nc = _compiled[niter]

    T = _build_table(dP)
    ptx, pty = _initial_pts(inds)

    in_maps = []
    for i in range(N_CORES):
        sl = slice(i * PTS_PER_CORE, (i + 1) * PTS_PER_CORE)
        in_maps.append({
            "tab": T,
            "p0x": ptx[sl].reshape(P, F),
            "p0y": pty[sl].reshape(P, F),
        })

    res = run_bass_kernel_spmd(nc, in_maps, list(range(N_CORES)))

    out = np.empty((2, NPTS), np.float32)
    for i in range(N_CORES):
        sl = slice(i * PTS_PER_CORE, (i + 1) * PTS_PER_CORE)
        out[0, sl] = res.results[i]["outy"].reshape(-1)
        out[1, sl] = res.results[i]["outx"].reshape(-1)
    return out
